# revision 19
# baseline (speedup 1.0000x reference)
"""GAT layer (nn_GAT_21930103013469) on 8 trn2 NeuronCores.

Reference (per batch b):
    Wh  = h @ W                                   [N, F]
    s1  = Wh @ a1,  s2 = Wh @ a2                  [N]
    e   = leakyrelu(s1[:,None] + s2[None,:], 0.2) [N, N]
    att = softmax(where(adj>0, e, -9e15), axis=0)   (normalized over ROWS i)
    out = elu(att @ Wh)

Strategy: data-parallel over B=16 (2 batches per core). Per batch the kernel
computes the attention matrix TRANSPOSED (PT[j, i]) so that
  - the softmax reduction (over i) is a free-dim reduction, fused into the
    ACT Exp pass via accum_out,
  - the output matmul out^T[o, i] = sum_j V[j, o] * PT[j, i] contracts j on
    partitions naturally.
The transposition happens on the tensor engine: adj is DMA-cast to fp8 {0,1}
and each 128x128 block is multiplied with a 128*I fp8 identity
(out = adj_blk^T @ 128I = 128*adjT), which lands the mask, pre-scaled, in
PSUM. A rank-1 f32r matmul accumulates s1[i] along the free dim, and the
per-partition s2[j] rides in as the ACT bias. Masked entries see
e - 128 (or e - 160 on the DVE route) before the leaky relu, so their
exp() is ~1e-9 relative - negligible vs the reference's exact 0.

out is produced transposed per batch ([F, N]); the host transposes back.
"""
import sys

sys.path.insert(0, "/opt/trn_rl_repo")

import numpy as np
import ml_dtypes

import concourse.bass as bass
import concourse.bacc as bacc
import concourse.tile as tile
from concourse import mybir
from concourse.bass_utils import run_bass_kernel_spmd

B, N, F = 16, 2048, 256
NCORES = 8
BPC = B // NCORES          # batches per core
NT = N // 128              # 16 i/j tiles
FT = F // 128              # 2 fin/fout tiles
ALPHA = 0.2
# number of (jt, half) units routed through the DVE-prelu path (0..32);
# the rest use ACT Prelu. Balances ACT vs DVE engine load.
N_DVE_ROUTE = 14

f32, f32r, bf16, fp8, i32 = (
    mybir.dt.float32, mybir.dt.float32r, mybir.dt.bfloat16,
    mybir.dt.float8e4, mybir.dt.int32,
)
f16 = mybir.dt.float16
SHIFT = 10.0        # global exponent shift: PT = exp(u - SHIFT) stays in fp16 range
VSCALE = 8.0        # combined with e^SHIFT from 1/Z', keeps V' in fp16 normal range
AF = mybir.ActivationFunctionType
OP = mybir.AluOpType


def build_nc(debug=False):
    nc = bacc.Bacc("TRN2", target_bir_lowering=False)
    h_d = nc.dram_tensor("h", [BPC, N, F], f32, kind="ExternalInput")
    adj_d = nc.dram_tensor("adj", [BPC, N, N], i32, kind="ExternalInput")
    W_d = nc.dram_tensor("w", [BPC, F, F], f32, kind="ExternalInput")
    a_d = nc.dram_tensor("a", [BPC, 2 * F, 1], f32, kind="ExternalInput")
    identB_d = nc.dram_tensor("identb", [128, 128], fp8, kind="ExternalInput")
    identC_d = nc.dram_tensor("identc", [128, 128], fp8, kind="ExternalInput")
    ident1_d = nc.dram_tensor("ident1", [128, 128], f32, kind="ExternalInput")
    out_d = nc.dram_tensor("out", [BPC, F, N], f32, kind="ExternalOutput")
    if debug:
        dbg_pt = nc.dram_tensor("dbg_pt", [128, N], f16, kind="ExternalOutput")
        dbg_z = nc.dram_tensor("dbg_z", [128, 1], f32, kind="ExternalOutput")
        dbg_v = nc.dram_tensor("dbg_v", [128, F], f16, kind="ExternalOutput")
        dbg_s1 = nc.dram_tensor("dbg_s1", [2, N], f32r, kind="ExternalOutput")

    with tile.TileContext(nc) as tc:
        with (
            tc.tile_pool(name="const", bufs=1) as const,
            tc.tile_pool(name="hin", bufs=4) as hin,
            tc.tile_pool(name="ht", bufs=1) as htp,
            tc.tile_pool(name="wa", bufs=1) as wa,
            tc.tile_pool(name="sc", bufs=2) as scp,
            tc.tile_pool(name="mnat", bufs=NT) as mnatp,
            tc.tile_pool(name="pt", bufs=NT) as ptp,
            tc.tile_pool(name="vs", bufs=NT) as vsp,
            tc.tile_pool(name="uu", bufs=2) as uup,
            tc.tile_pool(name="rw", bufs=2) as rwp,
            tc.tile_pool(name="zz", bufs=8) as zzp,
            tc.tile_pool(name="ep", bufs=2) as epp,
            tc.tile_pool(name="psS", bufs=1, space="PSUM") as psS,
            tc.tile_pool(name="psO", bufs=4, space="PSUM") as psO,
            tc.tile_pool(name="psM", bufs=2, space="PSUM") as psM,
        ):
            identB = const.tile([128, 128], fp8)
            nc.sync.dma_start(out=identB, in_=identB_d[:, :])
            identC = const.tile([128, 128], fp8)
            nc.sync.dma_start(out=identC, in_=identC_d[:, :])
            ident1 = const.tile([128, 128], f32)
            nc.sync.dma_start(out=ident1, in_=ident1_d[:, :])
            negshift = const.tile([128, 1], f32)
            nc.vector.memset(negshift, -SHIFT)
            ones_f = const.tile([2, 128], f32)
            nc.vector.memset(ones_f, 1.0)
            ones_r = const.tile([2, 128], f32r)
            nc.vector.tensor_copy(ones_r, ones_f)

            for b in range(BPC):
                # ---------- adj loads (DMA-cast int32 -> fp8 {0,1}) ----------
                mnat = []
                for it in range(NT):
                    m = mnatp.tile([128, N], fp8, tag="mnat")
                    nc.gpsimd.dma_start(
                        out=m, in_=adj_d[b, it * 128:(it + 1) * 128, :])
                    mnat.append(m)

                # ---------- prep: hT, W, a, c, s, Wh ----------
                hT = htp.tile([128, FT, N], f32r, tag="ht")
                for g in range(4):           # groups of 4 i-tiles
                    pht = [psM.tile([128, 512], f32, tag="psm",
                                    name=f"pht_{b}_{g}_{ft}") for ft in range(FT)]
                    for q in range(4):
                        it = 4 * g + q
                        hs = hin.tile([128, F], f32, tag="h", name=f"h_{b}_{it}")
                        nc.sync.dma_start(
                            out=hs, in_=h_d[b, it * 128:(it + 1) * 128, :])
                        for ft in range(FT):
                            # transpose h block [i,fin] -> [fin,i]
                            nc.tensor.transpose(
                                pht[ft][:, q * 128:(q + 1) * 128],
                                hs[:, ft * 128:(ft + 1) * 128],
                                ident1,
                            )
                    for ft in range(FT):
                        nc.vector.tensor_copy(
                            hT[:, ft, g * 512:(g + 1) * 512], pht[ft])

                Wsb = wa.tile([128, FT, F], f32, tag="w")
                nc.sync.dma_start(
                    out=Wsb, in_=W_d[b].rearrange("(kt p) o -> p kt o", p=128))
                Wr = wa.tile([128, FT, F], f32r, tag="wr")
                nc.vector.tensor_copy(Wr, Wsb)
                asb = wa.tile([128, 2, 2], f32, tag="a")
                for k in range(2):
                    for ot in range(2):
                        lo = k * 256 + ot * 128
                        nc.sync.dma_start(
                            out=asb[:, ot, k:k + 1],
                            in_=a_d[b, lo:lo + 128, :])

                # WT[o, fin] via PE transpose of W
                WT = wa.tile([128, FT, F], f32, tag="wt")
                for ot in range(FT):
                    pwt = psM.tile([128, 512], f32, tag="psm")
                    for kt in range(FT):
                        nc.tensor.transpose(
                            pwt[:, kt * 128:(kt + 1) * 128],
                            Wsb[:, kt, ot * 128:(ot + 1) * 128], ident1)
                    nc.vector.tensor_copy(WT[:, ot, :F], pwt[:, :F])

                # c[fin, 2] = W @ a  (contract o)
                csb = scp.tile([128, FT, 2], f32r, tag="c")
                for ft in range(FT):
                    pc = psM.tile([128, 512], f32, tag="psm")
                    for ot in range(FT):
                        nc.tensor.matmul(
                            pc[:, 0:2], WT[:, ot, ft * 128:(ft + 1) * 128],
                            asb[:, ot, :], start=(ot == 0), stop=(ot == FT - 1))
                    nc.vector.tensor_copy(csb[:, ft, :], pc[:, 0:2])

                # s_row[2, i] = c^T @ hT ; keep s1 row (f32r) for rank-1
                s1r = scp.tile([2, N], f32r, tag="s1r", bufs=1)
                s1r08 = scp.tile([2, N], f32r, tag="s1r08", bufs=1)
                s1lo = scp.tile([1, N], f32r, tag="s1lo", bufs=1)
                s1lo08 = scp.tile([1, N], f32r, tag="s1lo08", bufs=1)
                for ch in range(4):
                    sl = slice(ch * 512, (ch + 1) * 512)
                    ps = psM.tile([2, 512], f32, tag="psm")
                    for ft in range(FT):
                        nc.tensor.matmul(
                            ps, csb[:, ft, :], hT[:, ft, sl],
                            start=(ft == 0), stop=(ft == FT - 1))
                    # hi/lo split so the f32r rank-1 injects s1 at ~full fp32
                    # (engines can't write partition 1; stage lo rows and DMA)
                    nc.vector.tensor_copy(s1r[0:1, sl], ps[0:1, :])
                    nc.vector.scalar_tensor_tensor(
                        out=s1lo[0:1, sl], in0=s1r[0:1, sl], scalar=-1.0,
                        in1=ps[0:1, :], op0=OP.mult, op1=OP.add)
                    nc.vector.tensor_scalar_mul(s1r08[0:1, sl], ps[0:1, :], 0.8)
                    nc.vector.scalar_tensor_tensor(
                        out=s1lo08[0:1, sl], in0=ps[0:1, :], scalar=0.8,
                        in1=s1r08[0:1, sl], op0=OP.mult, op1=OP.subtract)
                nc.sync.dma_start(out=s1r[1:2, :], in_=s1lo[0:1, :])
                nc.sync.dma_start(out=s1r08[1:2, :], in_=s1lo08[0:1, :])

                # sT[i, 2] = hT^T @ c  (for per-partition s2)
                sT = scp.tile([128, NT, 2], f32, tag="st")
                for it in range(NT):
                    pst = psM.tile([128, 512], f32, tag="psm")
                    for ft in range(FT):
                        nc.tensor.matmul(
                            pst[:, 0:2], hT[:, ft, it * 128:(it + 1) * 128],
                            csb[:, ft, :], start=(ft == 0), stop=(ft == FT - 1))
                    nc.vector.tensor_copy(sT[:, it, :], pst[:, 0:2])

                # per-j-tile bias columns
                bias_act = scp.tile([128, NT], f32, tag="ba")
                nc.vector.tensor_scalar(
                    out=bias_act, in0=sT[:, :, 1], scalar1=1.0, scalar2=-128.0,
                    op0=OP.mult, op1=OP.add)
                bias_d1 = scp.tile([128, NT], f32, tag="b1")
                nc.vector.tensor_scalar(
                    out=bias_d1, in0=sT[:, :, 1], scalar1=0.8, scalar2=-64.0,
                    op0=OP.mult, op1=OP.add)
                bias_d2 = scp.tile([128, NT], f32, tag="b2")
                nc.vector.tensor_scalar(
                    out=bias_d2, in0=sT[:, :, 1], scalar1=0.2,
                    scalar2=-16.0 - SHIFT, op0=OP.mult, op1=OP.add)

                # ---------- j-sweep: build PT strips + Z + V ----------
                pts, vs = [], []
                for jt in range(NT):
                    pt = ptp.tile([128, N], f16, tag="pt", name=f"pt_{b}_{jt}")
                    zh = []
                    for hi in range(2):
                        route_dve = (jt * 2 + hi) % 32 < N_DVE_ROUTE
                        ident = identC if route_dve else identB
                        s1row = s1r08 if route_dve else s1r
                        S = psS.tile([128, 1024], f32, tag="S")
                        for q in range(8):
                            it = hi * 8 + q
                            nc.tensor.matmul(
                                S[:, q * 128:(q + 1) * 128],
                                mnat[it][:, jt * 128:(jt + 1) * 128],
                                ident, start=(q % 4 == 0), stop=False)
                        for c2 in range(2):
                            lo = hi * 1024 + c2 * 512
                            nc.tensor.matmul(
                                S[:, c2 * 512:(c2 + 1) * 512], ones_r,
                                s1row[0:2, lo:lo + 512], start=False, stop=True)
                        z = zzp.tile([128, 1], f32, tag="z")
                        if route_dve:
                            # r = relu(0.8*(e + (adjT-1)*160));  w = .25*S + r
                            # exp(w + 0.2*(s2-160)) == exp(lrelu(e+(adjT-1)*160))
                            r = rwp.tile([128, 1024], f32, tag="r")
                            nc.vector.tensor_scalar(
                                out=r, in0=S, scalar1=bias_d1[:, jt:jt + 1],
                                scalar2=0.0, op0=OP.add, op1=OP.max)
                            w = rwp.tile([128, 1024], f32, tag="w")
                            nc.vector.scalar_tensor_tensor(
                                out=w, in0=S, scalar=0.25, in1=r,
                                op0=OP.mult, op1=OP.add)
                            nc.scalar.activation(
                                out=pt[:, hi * 1024:(hi + 1) * 1024], in_=w,
                                func=AF.Exp, bias=bias_d2[:, jt:jt + 1],
                                scale=1.0, accum_out=z)
                        else:
                            u = uup.tile([128, 1024], f32, tag="u")
                            nc.scalar.activation(
                                out=u, in_=S, func=AF.Prelu,
                                bias=bias_act[:, jt:jt + 1], scale=1.0,
                                alpha=ALPHA)
                            nc.scalar.activation(
                                out=pt[:, hi * 1024:(hi + 1) * 1024], in_=u,
                                func=AF.Exp, bias=negshift, scale=1.0,
                                accum_out=z)
                        zh.append(z)
                    zs = zzp.tile([128, 1], f32, tag="zs")
                    nc.vector.tensor_add(zs, zh[0], zh[1])
                    zr = zzp.tile([128, 1], f32, tag="zr")
                    nc.vector.reciprocal(zr, zs)
                    zrv = zzp.tile([128, 1], f32, tag="zrv")
                    nc.vector.tensor_scalar_mul(zrv, zr, VSCALE)
                    pw = psM.tile([128, 512], f32, tag="psm",
                                  name=f"pw_{b}_{jt}")
                    for ft in range(FT):
                        nc.tensor.matmul(
                            pw[:, :F], hT[:, ft, jt * 128:(jt + 1) * 128],
                            Wr[:, ft, :], start=(ft == 0), stop=(ft == FT - 1))
                    v = vsp.tile([128, F], f16, tag="v", name=f"v_{b}_{jt}")
                    nc.vector.tensor_scalar_mul(v, pw[:, :F], zrv)
                    if debug and b == 0 and jt == 0:
                        nc.sync.dma_start(out=dbg_pt[:, :], in_=pt)
                        nc.sync.dma_start(out=dbg_z[:, :], in_=zs)
                        nc.sync.dma_start(out=dbg_v[:, :], in_=v)
                        nc.sync.dma_start(out=dbg_s1[:, :], in_=s1r)
                    pts.append(pt)
                    vs.append(v)

                # ---------- PV: out^T[o, i] = sum_j V[j,o] PT[j,i]; ELU ----------
                for hi in range(2):
                    Os = [psO.tile([128, 512], f32, tag="O",
                                   name=f"O_{b}_{hi}_{oc}") for oc in range(4)]
                    for jt in range(NT):
                        for ot in range(FT):
                            for c2 in range(2):
                                nc.tensor.matmul(
                                    Os[ot * 2 + c2],
                                    vs[jt][:, ot * 128:(ot + 1) * 128],
                                    pts[jt][:, hi * 1024 + c2 * 512:
                                            hi * 1024 + (c2 + 1) * 512],
                                    start=(jt == 0), stop=(jt == NT - 1))
                    for ot in range(FT):
                        for c2 in range(2):
                            O = Os[ot * 2 + c2]
                            ch = hi * 2 + c2
                            # elu(x) = relu(x) + exp(min(x,0)) - 1
                            r = epp.tile([128, 512], f32, tag="er")
                            nc.scalar.activation(out=r, in_=O, func=AF.Relu,
                                                 bias=0.0, scale=1.0 / VSCALE)
                            mn = epp.tile([128, 512], f32, tag="em")
                            nc.vector.scalar_tensor_tensor(
                                out=mn, in0=O, scalar=1.0 / VSCALE, in1=r,
                                op0=OP.mult, op1=OP.subtract)
                            t = epp.tile([128, 512], f32, tag="et")
                            nc.scalar.activation(out=t, in_=mn, func=AF.Exp,
                                                 bias=0.0, scale=1.0)
                            o_sb = epp.tile([128, 512], f32, tag="eo")
                            nc.vector.scalar_tensor_tensor(
                                out=o_sb, in0=t, scalar=-1.0, in1=r,
                                op0=OP.add, op1=OP.add)
                            nc.sync.dma_start(
                                out=out_d[b, ot * 128:(ot + 1) * 128,
                                          ch * 512:(ch + 1) * 512],
                                in_=o_sb)
    nc.compile()
    return nc


_NC_CACHE = {}


def _get_nc():
    if "nc" not in _NC_CACHE:
        _NC_CACHE["nc"] = build_nc()
    return _NC_CACHE["nc"]


def build_in_maps(h, adj, W, a):
    identB = (np.eye(128, dtype=np.float32) * 128.0).astype(ml_dtypes.float8_e4m3)
    identC = (np.eye(128, dtype=np.float32) * 64.0).astype(ml_dtypes.float8_e4m3)
    ident1 = np.eye(128, dtype=np.float32)
    in_maps = []
    for c in range(NCORES):
        sl = slice(c * BPC, (c + 1) * BPC)
        in_maps.append({
            "h": np.ascontiguousarray(h[sl]),
            "adj": np.ascontiguousarray(adj[sl]),
            "w": np.ascontiguousarray(W[sl]),
            "a": np.ascontiguousarray(a[sl]),
            "identb": identB,
            "identc": identC,
            "ident1": ident1,
        })
    return in_maps


def kernel(h, adj, W, a):
    nc = _get_nc()
    res = run_bass_kernel_spmd(nc, build_in_maps(h, adj, W, a),
                               list(range(NCORES)))
    outs = [np.asarray(r["out"]) for r in res.results]   # each [BPC, F, N]
    full = np.concatenate(outs, axis=0)                  # [B, F, N]
    return np.ascontiguousarray(full.transpose(0, 2, 1)).astype(np.float32)


# revision 20
# speedup vs baseline: 1.0147x; 1.0147x over previous
"""GAT layer (nn_GAT_21930103013469) on 8 trn2 NeuronCores.

Reference (per batch b):
    Wh  = h @ W                                   [N, F]
    s1  = Wh @ a1,  s2 = Wh @ a2                  [N]
    e   = leakyrelu(s1[:,None] + s2[None,:], 0.2) [N, N]
    att = softmax(where(adj>0, e, -9e15), axis=0)   (normalized over ROWS i)
    out = elu(att @ Wh)

Strategy: data-parallel over B=16 (2 batches per core). Per batch the kernel
computes the attention matrix TRANSPOSED (PT[j, i]) so that
  - the softmax reduction (over i) is a free-dim reduction, fused into the
    ACT Exp pass via accum_out,
  - the output matmul out^T[o, i] = sum_j V[j, o] * PT[j, i] contracts j on
    partitions naturally.
The transposition happens on the tensor engine: adj is DMA-cast to fp8 {0,1}
and each 128x128 block is multiplied with a 128*I fp8 identity
(out = adj_blk^T @ 128I = 128*adjT), which lands the mask, pre-scaled, in
PSUM. A rank-1 f32r matmul accumulates s1[i] along the free dim, and the
per-partition s2[j] rides in as the ACT bias. Masked entries see
e - 128 (or e - 160 on the DVE route) before the leaky relu, so their
exp() is ~1e-9 relative - negligible vs the reference's exact 0.

out is produced transposed per batch ([F, N]); the host transposes back.
"""
import sys

sys.path.insert(0, "/opt/trn_rl_repo")

import numpy as np
import ml_dtypes

import concourse.bass as bass
import concourse.bacc as bacc
import concourse.tile as tile
from concourse import mybir
from concourse.bass_utils import run_bass_kernel_spmd

B, N, F = 16, 2048, 256
NCORES = 8
BPC = B // NCORES          # batches per core
NT = N // 128              # 16 i/j tiles
FT = F // 128              # 2 fin/fout tiles
ALPHA = 0.2
# number of (jt, half) units routed through the DVE-prelu path (0..32);
# the rest use ACT Prelu. Balances ACT vs DVE engine load.
N_DVE_ROUTE = 14

f32, f32r, bf16, fp8, i32 = (
    mybir.dt.float32, mybir.dt.float32r, mybir.dt.bfloat16,
    mybir.dt.float8e4, mybir.dt.int32,
)
f16 = mybir.dt.float16
SHIFT = 10.0        # global exponent shift: PT = exp(u - SHIFT) stays in fp16 range
VSCALE = 8.0        # combined with e^SHIFT from 1/Z', keeps V' in fp16 normal range
AF = mybir.ActivationFunctionType
OP = mybir.AluOpType


def build_nc(debug=False):
    nc = bacc.Bacc("TRN2", target_bir_lowering=False)
    h_d = nc.dram_tensor("h", [BPC, N, F], f32, kind="ExternalInput")
    adj_d = nc.dram_tensor("adj", [BPC, N, N], i32, kind="ExternalInput")
    W_d = nc.dram_tensor("w", [BPC, F, F], f32, kind="ExternalInput")
    a_d = nc.dram_tensor("a", [BPC, 2 * F, 1], f32, kind="ExternalInput")
    identB_d = nc.dram_tensor("identb", [128, 128], fp8, kind="ExternalInput")
    identC_d = nc.dram_tensor("identc", [128, 128], fp8, kind="ExternalInput")
    ident1_d = nc.dram_tensor("ident1", [128, 128], f32, kind="ExternalInput")
    out_d = nc.dram_tensor("out", [BPC, F, N], f32, kind="ExternalOutput")
    if debug:
        dbg_pt = nc.dram_tensor("dbg_pt", [128, N], f16, kind="ExternalOutput")
        dbg_z = nc.dram_tensor("dbg_z", [128, 1], f32, kind="ExternalOutput")
        dbg_v = nc.dram_tensor("dbg_v", [128, F], f16, kind="ExternalOutput")
        dbg_s1 = nc.dram_tensor("dbg_s1", [2, N], f32r, kind="ExternalOutput")

    with tile.TileContext(nc) as tc:
        with (
            tc.tile_pool(name="const", bufs=1) as const,
            tc.tile_pool(name="hin", bufs=4) as hin,
            tc.tile_pool(name="ht", bufs=1) as htp,
            tc.tile_pool(name="wa", bufs=1) as wa,
            tc.tile_pool(name="sc", bufs=2) as scp,
            tc.tile_pool(name="mnat", bufs=NT) as mnatp,
            tc.tile_pool(name="pt", bufs=NT) as ptp,
            tc.tile_pool(name="vs", bufs=NT) as vsp,
            tc.tile_pool(name="uu", bufs=2) as uup,
            tc.tile_pool(name="rw", bufs=2) as rwp,
            tc.tile_pool(name="zz", bufs=8) as zzp,
            tc.tile_pool(name="ep", bufs=2) as epp,
            tc.tile_pool(name="psS", bufs=2, space="PSUM") as psS,
            tc.tile_pool(name="psO", bufs=3, space="PSUM") as psO,
            tc.tile_pool(name="psM", bufs=1, space="PSUM") as psM,
        ):
            identB = const.tile([128, 128], fp8)
            nc.sync.dma_start(out=identB, in_=identB_d[:, :])
            identC = const.tile([128, 128], fp8)
            nc.sync.dma_start(out=identC, in_=identC_d[:, :])
            ident1 = const.tile([128, 128], f32)
            nc.sync.dma_start(out=ident1, in_=ident1_d[:, :])
            negshift = const.tile([128, 1], f32)
            nc.vector.memset(negshift, -SHIFT)
            ones_f = const.tile([2, 128], f32)
            nc.vector.memset(ones_f, 1.0)
            ones_r = const.tile([2, 128], f32r)
            nc.vector.tensor_copy(ones_r, ones_f)

            for b in range(BPC):
                # ---------- adj loads (DMA-cast int32 -> fp8 {0,1}) ----------
                mnat = []
                for it in range(NT):
                    m = mnatp.tile([128, N], fp8, tag="mnat")
                    nc.gpsimd.dma_start(
                        out=m, in_=adj_d[b, it * 128:(it + 1) * 128, :])
                    mnat.append(m)

                # ---------- prep: hT, W, a, c, s, Wh ----------
                hT = htp.tile([128, FT, N], f32r, tag="ht")
                for g in range(4):           # groups of 4 i-tiles
                    hss = []
                    for q in range(4):
                        it = 4 * g + q
                        hs = hin.tile([128, F], f32, tag="h", name=f"h_{b}_{it}")
                        nc.sync.dma_start(
                            out=hs, in_=h_d[b, it * 128:(it + 1) * 128, :])
                        hss.append(hs)
                    for ft in range(FT):
                        pht = psM.tile([128, 512], f32, tag="psm",
                                       name=f"pht_{b}_{g}_{ft}")
                        for q in range(4):
                            # transpose h block [i,fin] -> [fin,i]
                            nc.tensor.transpose(
                                pht[:, q * 128:(q + 1) * 128],
                                hss[q][:, ft * 128:(ft + 1) * 128],
                                ident1,
                            )
                        nc.vector.tensor_copy(
                            hT[:, ft, g * 512:(g + 1) * 512], pht)

                Wsb = wa.tile([128, FT, F], f32, tag="w")
                nc.sync.dma_start(
                    out=Wsb, in_=W_d[b].rearrange("(kt p) o -> p kt o", p=128))
                Wr = wa.tile([128, FT, F], f32r, tag="wr")
                nc.vector.tensor_copy(Wr, Wsb)
                asb = wa.tile([128, 2, 2], f32, tag="a")
                for k in range(2):
                    for ot in range(2):
                        lo = k * 256 + ot * 128
                        nc.sync.dma_start(
                            out=asb[:, ot, k:k + 1],
                            in_=a_d[b, lo:lo + 128, :])

                # WT[o, fin] via PE transpose of W
                WT = wa.tile([128, FT, F], f32, tag="wt")
                for ot in range(FT):
                    pwt = psM.tile([128, 512], f32, tag="psm")
                    for kt in range(FT):
                        nc.tensor.transpose(
                            pwt[:, kt * 128:(kt + 1) * 128],
                            Wsb[:, kt, ot * 128:(ot + 1) * 128], ident1)
                    nc.vector.tensor_copy(WT[:, ot, :F], pwt[:, :F])

                # c[fin, 2] = W @ a  (contract o)
                csb = scp.tile([128, FT, 2], f32r, tag="c")
                for ft in range(FT):
                    pc = psM.tile([128, 512], f32, tag="psm")
                    for ot in range(FT):
                        nc.tensor.matmul(
                            pc[:, 0:2], WT[:, ot, ft * 128:(ft + 1) * 128],
                            asb[:, ot, :], start=(ot == 0), stop=(ot == FT - 1))
                    nc.vector.tensor_copy(csb[:, ft, :], pc[:, 0:2])

                # s_row[2, i] = c^T @ hT ; keep s1 row (f32r) for rank-1
                s1r = scp.tile([2, N], f32r, tag="s1r", bufs=1)
                s1r08 = scp.tile([2, N], f32r, tag="s1r08", bufs=1)
                s1lo = scp.tile([1, N], f32r, tag="s1lo", bufs=1)
                s1lo08 = scp.tile([1, N], f32r, tag="s1lo08", bufs=1)
                for ch in range(4):
                    sl = slice(ch * 512, (ch + 1) * 512)
                    ps = psM.tile([2, 512], f32, tag="psm")
                    for ft in range(FT):
                        nc.tensor.matmul(
                            ps, csb[:, ft, :], hT[:, ft, sl],
                            start=(ft == 0), stop=(ft == FT - 1))
                    # hi/lo split so the f32r rank-1 injects s1 at ~full fp32
                    # (engines can't write partition 1; stage lo rows and DMA)
                    nc.vector.tensor_copy(s1r[0:1, sl], ps[0:1, :])
                    nc.vector.scalar_tensor_tensor(
                        out=s1lo[0:1, sl], in0=s1r[0:1, sl], scalar=-1.0,
                        in1=ps[0:1, :], op0=OP.mult, op1=OP.add)
                    nc.vector.tensor_scalar_mul(s1r08[0:1, sl], ps[0:1, :], 0.8)
                    nc.vector.scalar_tensor_tensor(
                        out=s1lo08[0:1, sl], in0=ps[0:1, :], scalar=0.8,
                        in1=s1r08[0:1, sl], op0=OP.mult, op1=OP.subtract)
                nc.sync.dma_start(out=s1r[1:2, :], in_=s1lo[0:1, :])
                nc.sync.dma_start(out=s1r08[1:2, :], in_=s1lo08[0:1, :])

                # sT[i, 2] = hT^T @ c  (for per-partition s2)
                sT = scp.tile([128, NT, 2], f32, tag="st")
                for it in range(NT):
                    pst = psM.tile([128, 512], f32, tag="psm")
                    for ft in range(FT):
                        nc.tensor.matmul(
                            pst[:, 0:2], hT[:, ft, it * 128:(it + 1) * 128],
                            csb[:, ft, :], start=(ft == 0), stop=(ft == FT - 1))
                    nc.vector.tensor_copy(sT[:, it, :], pst[:, 0:2])

                # per-j-tile bias columns
                bias_act = scp.tile([128, NT], f32, tag="ba")
                nc.vector.tensor_scalar(
                    out=bias_act, in0=sT[:, :, 1], scalar1=1.0, scalar2=-128.0,
                    op0=OP.mult, op1=OP.add)
                bias_d1 = scp.tile([128, NT], f32, tag="b1")
                nc.vector.tensor_scalar(
                    out=bias_d1, in0=sT[:, :, 1], scalar1=0.8, scalar2=-64.0,
                    op0=OP.mult, op1=OP.add)
                bias_d2 = scp.tile([128, NT], f32, tag="b2")
                nc.vector.tensor_scalar(
                    out=bias_d2, in0=sT[:, :, 1], scalar1=0.2,
                    scalar2=-16.0 - SHIFT, op0=OP.mult, op1=OP.add)

                # ---------- j-sweep: build PT strips + Z + V ----------
                pts, vs = [], []
                for jt in range(NT):
                    pt = ptp.tile([128, N], f16, tag="pt", name=f"pt_{b}_{jt}")
                    zh = []
                    for hi in range(2):
                        route_dve = (jt * 2 + hi) % 32 < N_DVE_ROUTE
                        ident = identC if route_dve else identB
                        s1row = s1r08 if route_dve else s1r
                        S = psS.tile([128, 1024], f32, tag="S")
                        for q in range(8):
                            it = hi * 8 + q
                            nc.tensor.matmul(
                                S[:, q * 128:(q + 1) * 128],
                                mnat[it][:, jt * 128:(jt + 1) * 128],
                                ident, start=(q % 4 == 0), stop=False)
                        for c2 in range(2):
                            lo = hi * 1024 + c2 * 512
                            nc.tensor.matmul(
                                S[:, c2 * 512:(c2 + 1) * 512], ones_r,
                                s1row[0:2, lo:lo + 512], start=False, stop=True)
                        z = zzp.tile([128, 1], f32, tag="z")
                        if route_dve:
                            # r = relu(0.8*(e + (adjT-1)*160));  w = .25*S + r
                            # exp(w + 0.2*(s2-160)) == exp(lrelu(e+(adjT-1)*160))
                            r = rwp.tile([128, 1024], f32, tag="r")
                            nc.vector.tensor_scalar(
                                out=r, in0=S, scalar1=bias_d1[:, jt:jt + 1],
                                scalar2=0.0, op0=OP.add, op1=OP.max)
                            w = rwp.tile([128, 1024], f32, tag="w")
                            nc.vector.scalar_tensor_tensor(
                                out=w, in0=S, scalar=0.25, in1=r,
                                op0=OP.mult, op1=OP.add)
                            nc.scalar.activation(
                                out=pt[:, hi * 1024:(hi + 1) * 1024], in_=w,
                                func=AF.Exp, bias=bias_d2[:, jt:jt + 1],
                                scale=1.0, accum_out=z)
                        else:
                            u = uup.tile([128, 1024], f32, tag="u")
                            nc.scalar.activation(
                                out=u, in_=S, func=AF.Prelu,
                                bias=bias_act[:, jt:jt + 1], scale=1.0,
                                alpha=ALPHA)
                            nc.scalar.activation(
                                out=pt[:, hi * 1024:(hi + 1) * 1024], in_=u,
                                func=AF.Exp, bias=negshift, scale=1.0,
                                accum_out=z)
                        zh.append(z)
                    zs = zzp.tile([128, 1], f32, tag="zs")
                    nc.vector.tensor_add(zs, zh[0], zh[1])
                    zr = zzp.tile([128, 1], f32, tag="zr")
                    nc.vector.reciprocal(zr, zs)
                    zrv = zzp.tile([128, 1], f32, tag="zrv")
                    nc.vector.tensor_scalar_mul(zrv, zr, VSCALE)
                    pw = psM.tile([128, 512], f32, tag="psm",
                                  name=f"pw_{b}_{jt}")
                    for ft in range(FT):
                        nc.tensor.matmul(
                            pw[:, :F], hT[:, ft, jt * 128:(jt + 1) * 128],
                            Wr[:, ft, :], start=(ft == 0), stop=(ft == FT - 1))
                    v = vsp.tile([128, F], f16, tag="v", name=f"v_{b}_{jt}")
                    nc.vector.tensor_scalar_mul(v, pw[:, :F], zrv)
                    if debug and b == 0 and jt == 0:
                        nc.sync.dma_start(out=dbg_pt[:, :], in_=pt)
                        nc.sync.dma_start(out=dbg_z[:, :], in_=zs)
                        nc.sync.dma_start(out=dbg_v[:, :], in_=v)
                        nc.sync.dma_start(out=dbg_s1[:, :], in_=s1r)
                    pts.append(pt)
                    vs.append(v)

                # ---------- PV: out^T[o, i] = sum_j V[j,o] PT[j,i]; ELU ----------
                for hi in range(2):
                    for ot in range(FT):
                        Os = [psO.tile([128, 512], f32, tag="O",
                                       name=f"O_{b}_{hi}_{ot}_{oc}")
                              for oc in range(2)]
                        for jt in range(NT):
                            for c2 in range(2):
                                nc.tensor.matmul(
                                    Os[c2],
                                    vs[jt][:, ot * 128:(ot + 1) * 128],
                                    pts[jt][:, hi * 1024 + c2 * 512:
                                            hi * 1024 + (c2 + 1) * 512],
                                    start=(jt == 0), stop=(jt == NT - 1))
                        for c2 in range(2):
                            O = Os[c2]
                            ch = hi * 2 + c2
                            # elu(x) = relu(x) + exp(min(x,0)) - 1
                            r = epp.tile([128, 512], f32, tag="er")
                            nc.scalar.activation(out=r, in_=O, func=AF.Relu,
                                                 bias=0.0, scale=1.0 / VSCALE)
                            mn = epp.tile([128, 512], f32, tag="em")
                            nc.vector.scalar_tensor_tensor(
                                out=mn, in0=O, scalar=1.0 / VSCALE, in1=r,
                                op0=OP.mult, op1=OP.subtract)
                            t = epp.tile([128, 512], f32, tag="et")
                            nc.scalar.activation(out=t, in_=mn, func=AF.Exp,
                                                 bias=0.0, scale=1.0)
                            o_sb = epp.tile([128, 512], f32, tag="eo")
                            nc.vector.scalar_tensor_tensor(
                                out=o_sb, in0=t, scalar=-1.0, in1=r,
                                op0=OP.add, op1=OP.add)
                            nc.sync.dma_start(
                                out=out_d[b, ot * 128:(ot + 1) * 128,
                                          ch * 512:(ch + 1) * 512],
                                in_=o_sb)
    nc.compile()
    return nc


_NC_CACHE = {}


def _get_nc():
    if "nc" not in _NC_CACHE:
        _NC_CACHE["nc"] = build_nc()
    return _NC_CACHE["nc"]


def build_in_maps(h, adj, W, a):
    identB = (np.eye(128, dtype=np.float32) * 128.0).astype(ml_dtypes.float8_e4m3)
    identC = (np.eye(128, dtype=np.float32) * 64.0).astype(ml_dtypes.float8_e4m3)
    ident1 = np.eye(128, dtype=np.float32)
    in_maps = []
    for c in range(NCORES):
        sl = slice(c * BPC, (c + 1) * BPC)
        in_maps.append({
            "h": np.ascontiguousarray(h[sl]),
            "adj": np.ascontiguousarray(adj[sl]),
            "w": np.ascontiguousarray(W[sl]),
            "a": np.ascontiguousarray(a[sl]),
            "identb": identB,
            "identc": identC,
            "ident1": ident1,
        })
    return in_maps


def kernel(h, adj, W, a):
    nc = _get_nc()
    res = run_bass_kernel_spmd(nc, build_in_maps(h, adj, W, a),
                               list(range(NCORES)))
    outs = [np.asarray(r["out"]) for r in res.results]   # each [BPC, F, N]
    full = np.concatenate(outs, axis=0)                  # [B, F, N]
    return np.ascontiguousarray(full.transpose(0, 2, 1)).astype(np.float32)
